# revision 7
# baseline (speedup 1.0000x reference)
"""Expert-parallel CMoE kernel for 8 Trainium2 NeuronCores (v2).

Sharding (hardcoded for B=8, T=2048, D=1024, F=2048, E=16, C=1024):
  core k owns batch k (token shift, receptance, output) and experts
  {2k, 2k+1} (FFN). Hash routing is int math on token_ids, done on host;
  the resulting permutations ship to the cores as index tensors.

Schedule per core (PE-centric; everything else hides behind matmuls):
  phase A  (~35us, PE idle, DMA/DVE-bound): stream x once per tile,
           token-shift via partition-offset copies (no second HBM load),
           all-bf16 vector math; scatter xk rows into the single
           dispatch buffer; store xr chunks for the receptance phase.
  dispatch: ONE AllToAll (4.7 MB) fired at loop end; it runs on the
           collective engine while the PE does the receptance.
  receptance (~68us PE): per 512-token chunk, transposing dma_gather of
           xr, 64 matmuls, sigmoid, store r rows to DRAM.
  phase C (~275us PE): per expert el in {0,1}: transposing gathers of
           the expert queue, FFN1 (relu^2), FFN2 split by output
           column-half; each (el, half) fires its own combine AllToAll
           (1.45 MB) so only the last one is exposed.
  phase D  (interleaved): per (el, half): contiguous load of the recv
           buffer, multiply by gathered r rows, indirect-scatter fp32
           rows into out[token]. Outputs are two [T+1, 512] tensors
           (indirect DMA needs offset-0 APs); row T is a trash row for
           pad slots; dropped tokens keep the zero-init value.
All matmuls bf16 with fp32 PSUM accumulation.
"""
import sys

for _p in ("/opt/trn_rl_repo", "/root/.axon_site/_ro/trn_rl_repo"):
    if _p not in sys.path:
        sys.path.append(_p)

import numpy as np
import ml_dtypes

import concourse.bass as bass
import concourse.bacc as bacc
import concourse.mybir as mybir
import concourse.tile as tile
from concourse.bass_utils import run_bass_kernel_spmd

P = 128
B, T, D, F, E = 8, 2048, 1024, 2048, 16
N = B * T
C = max(4, N // E)          # 1024
HASH_PRIME = 5099
NCORES = 8
EPC = E // NCORES           # experts per core = 2
DC = D // P                 # 8
FC = F // P                 # 16
BF16 = mybir.dt.bfloat16
F32 = mybir.dt.float32
I16 = mybir.dt.int16
I32 = mybir.dt.int32
nbf16 = ml_dtypes.bfloat16
AF = mybir.ActivationFunctionType

_CACHE = {}


def _r16(v):
    return int(-(-int(v) // 16) * 16)


def _wrap16(a):
    a = np.asarray(a, np.int16)
    w = a.reshape(-1, 16).T.copy()       # j at [j%16, j//16]
    return np.tile(w, (8, 1))            # replicated across 8 Q7 cores


def _route(token_ids):
    tid = np.asarray(token_ids).reshape(N).astype(np.int64)
    e = (tid * HASH_PRIME) % E
    onehot = (e[:, None] == np.arange(E)).astype(np.int64)
    pos = onehot.cumsum(0)[np.arange(N), e] - 1
    keep = pos < C
    return e, pos, keep


def _build_indices(token_ids):
    e, pos, keep = _route(token_ids)
    n_idx = np.arange(N)
    src = n_idx // T
    dst = e // EPC
    el = e % EPC
    kept = n_idx[keep]

    # dispatch: single A2A; rank within (src,dst) in n order
    cntd = np.zeros((NCORES, NCORES), np.int64)
    rankd = np.zeros(N, np.int64)
    for n in kept:
        rankd[n] = cntd[src[n], dst[n]]
        cntd[src[n], dst[n]] += 1
    KD = _r16(cntd.max())
    ZRD = NCORES * KD
    slot = np.where(keep, dst * KD + rankd, ZRD)

    recv_row = np.full((NCORES, EPC * C), ZRD, np.int64)
    for n in kept:
        recv_row[dst[n], el[n] * C + pos[n]] = src[n] * KD + rankd[n]

    # combine: 2 row-chunks by el; A2As additionally split by col half
    cellcnt = np.zeros((NCORES, NCORES, EPC), np.int64)
    rankc = np.zeros(N, np.int64)
    for n in kept:
        rankc[n] = cellcnt[dst[n], src[n], el[n]]
        cellcnt[dst[n], src[n], el[n]] += 1
    KC = _r16(cellcnt.max())
    NBR = NCORES * KC
    ZRC = NBR

    s2 = np.full((NCORES, EPC, C), ZRC, np.int64)
    for n in kept:
        s2[dst[n], el[n], pos[n]] = src[n] * KC + rankc[n]

    rg_idx = np.zeros((NCORES, EPC, NBR), np.int64)
    os_idx = np.full((NCORES, EPC, NBR), T, np.int64)
    for n in kept:
        h = src[n]
        lt = n - h * T
        row = dst[n] * KC + rankc[n]
        rg_idx[h, el[n], row] = lt
        os_idx[h, el[n], row] = lt

    NB = NBR // P
    per_core = []
    for k in range(NCORES):
        tok = slice(k * T, (k + 1) * T)
        per_core.append({
            "srcD32": slot[tok].astype(np.int32).reshape(T // P, P).T.copy(),
            "slot16": _wrap16(recv_row[k]),
            "s2_32": np.concatenate(
                [s2[k, eli].reshape(C // P, P).T for eli in range(EPC)],
                axis=1).astype(np.int32).copy(),
            "rg16": np.concatenate(
                [_wrap16(rg_idx[k, eli]) for eli in range(EPC)], axis=1),
            "os32": np.concatenate(
                [os_idx[k, eli].reshape(NB, P).T for eli in range(EPC)],
                axis=1).astype(np.int32).copy(),
        })
    return (KD, KC), per_core


def _build_nc(cfg):
    KD, KC = cfg
    RD = NCORES * KD
    NBR = NCORES * KC
    NB = NBR // P
    HD = D // 2              # 512, output column half

    nc = bacc.Bacc("TRN2", target_bir_lowering=False, debug=False,
                   num_devices=NCORES)

    x_ext = nc.dram_tensor("x_ext", [T + 1, D], F32, kind="ExternalInput")
    maa_k = nc.dram_tensor("maa_k", [1, D], BF16, kind="ExternalInput")
    maa_r = nc.dram_tensor("maa_r", [1, D], BF16, kind="ExternalInput")
    wrt = nc.dram_tensor("wrt", [D, D], BF16, kind="ExternalInput")
    wk = nc.dram_tensor("wk", [EPC, D, F], BF16, kind="ExternalInput")
    wv = nc.dram_tensor("wv", [EPC, F, D], BF16, kind="ExternalInput")
    srcD32 = nc.dram_tensor("srcD32", [P, T // P], I32, kind="ExternalInput")
    slot16 = nc.dram_tensor("slot16", [P, EPC * C // 16], I16,
                            kind="ExternalInput")
    s2_32 = nc.dram_tensor("s2_32", [P, EPC * (C // P)], I32,
                           kind="ExternalInput")
    rg16 = nc.dram_tensor("rg16", [P, EPC * (NBR // 16)], I16,
                          kind="ExternalInput")
    os32 = nc.dram_tensor("os32", [P, EPC * NB], I32, kind="ExternalInput")
    iota16 = nc.dram_tensor("iota16", [P, 512 // 16], I16,
                            kind="ExternalInput")
    out0 = nc.dram_tensor("out0", [T + 1, HD], F32, kind="ExternalOutput")
    out1 = nc.dram_tensor("out1", [T + 1, HD], F32, kind="ExternalOutput")
    outs = [out0, out1]

    rg = [list(range(NCORES))]

    with tile.TileContext(nc) as tc:
        with (
            tc.tile_pool(name="dram", bufs=1, space="DRAM") as dram,
            tc.tile_pool(name="misc", bufs=1) as misc,
        ):
            a1_in = dram.tile([RD + 1, D], BF16)
            recv1 = dram.tile([RD + 1, D], BF16)
            a2h = [[dram.tile([NBR + 1, HD], BF16, name=f"a2_{eli}_{hf}")
                    for hf in range(2)] for eli in range(EPC)]
            recv2h = [[dram.tile([NBR, HD], BF16, name=f"rc2_{eli}_{hf}")
                       for hf in range(2)] for eli in range(EPC)]
            xr_bufs = [dram.tile([512, D], BF16, name=f"xr_buf{i}")
                       for i in range(4)]
            r_dram = dram.tile([T, D], BF16)

            # small index/constant loads (sync queue, ~100 KB total)
            srcDsb = misc.tile([P, T // P], I32)
            nc.sync.dma_start(out=srcDsb[:], in_=srcD32[:])
            maakb = misc.tile([P, D], BF16)
            nc.sync.dma_start(out=maakb[:], in_=maa_k[:].to_broadcast([P, D]))
            maarb = misc.tile([P, D], BF16)
            nc.sync.dma_start(out=maarb[:], in_=maa_r[:].to_broadcast([P, D]))
            sl16 = misc.tile([P, EPC * C // 16], I16)
            nc.sync.dma_start(out=sl16[:], in_=slot16[:])
            s2sb = misc.tile([P, EPC * (C // P)], I32)
            nc.sync.dma_start(out=s2sb[:], in_=s2_32[:])
            rgsb = misc.tile([P, EPC * (NBR // 16)], I16)
            nc.sync.dma_start(out=rgsb[:], in_=rg16[:])
            ossb = misc.tile([P, EPC * NB], I32)
            nc.sync.dma_start(out=ossb[:], in_=os32[:])
            io16 = misc.tile([P, 512 // 16], I16)
            nc.sync.dma_start(out=io16[:], in_=iota16[:])

            zrow = misc.tile([1, D], BF16)
            nc.vector.memzero(zrow[:])
            nc.scalar.dma_start(out=recv1[RD:RD + 1, :], in_=zrow[:])

            with tc.tile_pool(name="pw0", bufs=1) as pw0:
                # receptance weights: SWDGE load so it never blocks the
                # sync-queue x stream
                wrt_sb = pw0.tile([P, DC, D], BF16)
                nc.sync.dma_start(
                    out=wrt_sb[:], in_=wrt.rearrange("(c p) e -> p c e", p=P))

                # ---- phase A: token shift, dispatch scatter, xr spill.
                # xp is a second HBM load at -1 row offset (SBUF->SBUF
                # partition-shift DMAs risk the transpose-DMA deadlock).
                with tc.tile_pool(name="pa", bufs=4) as pa:
                    for t in range(T // P):
                        xc = pa.tile([P, D], F32, tag="xc")
                        nc.sync.dma_start(
                            out=xc[:],
                            in_=x_ext[1 + t * P:1 + (t + 1) * P, :])
                        xp = pa.tile([P, D], F32, tag="xp")
                        nc.sync.dma_start(
                            out=xp[:], in_=x_ext[t * P:(t + 1) * P, :])
                        xcb = pa.tile([P, D], BF16, tag="xcb")
                        nc.scalar.activation(out=xcb[:], in_=xc[:],
                                             func=AF.Copy)
                        dx = pa.tile([P, D], BF16, tag="dx")
                        nc.vector.tensor_sub(out=dx[:], in0=xp[:], in1=xc[:])
                        tmpk = pa.tile([P, D], BF16, tag="tmpk")
                        nc.vector.tensor_mul(out=tmpk[:], in0=dx[:],
                                             in1=maakb[:])
                        xk = pa.tile([P, D], BF16, tag="xk")
                        nc.vector.tensor_add(out=xk[:], in0=tmpk[:],
                                             in1=xcb[:])
                        nc.gpsimd.indirect_dma_start(
                            out=a1_in[:],
                            out_offset=bass.IndirectOffsetOnAxis(
                                ap=srcDsb[:, t:t + 1], axis=0),
                            in_=xk[:], in_offset=None)
                        tmpr = pa.tile([P, D], BF16, tag="tmpr")
                        nc.vector.tensor_mul(out=tmpr[:], in0=dx[:],
                                             in1=maarb[:])
                        xr = pa.tile([P, D], BF16, tag="xr")
                        nc.vector.tensor_add(out=xr[:], in0=tmpr[:],
                                             in1=xcb[:])
                        nc.scalar.dma_start(
                            out=xr_bufs[t // 4][(t % 4) * P:(t % 4 + 1) * P, :],
                            in_=xr[:])

                # ---- dispatch A2A (runs on CC silicon during receptance)
                nc.gpsimd.collective_compute(
                    "AllToAll", mybir.AluOpType.bypass, replica_groups=rg,
                    ins=[a1_in[0:RD, :]], outs=[recv1[0:RD, :]])

                # expert weights for el=0: prefetch during receptance

                # ---- receptance: r = sigmoid(xr @ wrt), 512 tokens/chunk
                with (
                    tc.tile_pool(name="prx", bufs=2) as prx,
                    tc.tile_pool(name="psr", bufs=2, space="PSUM") as psr,
                ):
                    for ck in range(4):
                        xrT = prx.tile([P, DC, 512], BF16, tag="xrT")
                        nc.gpsimd.dma_gather(
                            out_ap=xrT[:], in_ap=xr_bufs[ck][:],
                            idxs_ap=io16[:, 0:32],
                            num_idxs=512, num_idxs_reg=512, elem_size=D,
                            transpose=True)
                        for tt in range(4):
                            pr0 = psr.tile([P, 512], F32, space="PSUM",
                                           tag="pr0")
                            pr1 = psr.tile([P, 512], F32, space="PSUM",
                                           tag="pr1")
                            for dc in range(DC):
                                nc.tensor.matmul(
                                    out=pr0[:],
                                    lhsT=xrT[:, dc, tt * P:(tt + 1) * P],
                                    rhs=wrt_sb[:, dc, 0:512],
                                    start=(dc == 0), stop=(dc == DC - 1))
                                nc.tensor.matmul(
                                    out=pr1[:],
                                    lhsT=xrT[:, dc, tt * P:(tt + 1) * P],
                                    rhs=wrt_sb[:, dc, 512:1024],
                                    start=(dc == 0), stop=(dc == DC - 1))
                            rsb = prx.tile([P, D], BF16, tag="rsb")
                            nc.scalar.activation(out=rsb[:, 0:512],
                                                 in_=pr0[:], func=AF.Sigmoid)
                            nc.scalar.activation(out=rsb[:, 512:1024],
                                                 in_=pr1[:], func=AF.Sigmoid)
                            r0 = ck * 512 + tt * P
                            nc.scalar.dma_start(out=r_dram[r0:r0 + P, :],
                                                in_=rsb[:])

            # ---------------- phase C: expert FFNs + combine + phase D
            with (
                tc.tile_pool(name="pwk", bufs=2) as pwk,
                tc.tile_pool(name="pwv", bufs=1) as pwv,
                tc.tile_pool(name="pfx", bufs=1) as pfx,
                tc.tile_pool(name="pfh", bufs=1) as pfh,
                tc.tile_pool(name="pfhr", bufs=2) as pfhr,
                tc.tile_pool(name="pfy", bufs=2) as pfy,
                tc.tile_pool(name="prf", bufs=1) as prf,
                tc.tile_pool(name="pdl", bufs=1) as pdl,
                tc.tile_pool(name="pdo", bufs=2) as pdo,
                tc.tile_pool(name="psh", bufs=2, space="PSUM") as psh,
                tc.tile_pool(name="psy", bufs=2, space="PSUM") as psy,
            ):
                def d_chunk(eli, hf):
                    """recv2h[eli][hf] -> out[hf] rows (after its A2A)."""
                    yrecv = pdl.tile([P, NB, HD], BF16, tag="yrecv")
                    nc.sync.dma_start(
                        out=yrecv[:],
                        in_=recv2h[eli][hf].rearrange("(b p) d -> p b d", p=P))
                    rfel = rf[eli]
                    for b in range(NB):
                        yo = pdo.tile([P, HD], F32, tag="yo")
                        nc.vector.tensor_mul(
                            out=yo[:], in0=yrecv[:, b, :],
                            in1=rfel[:, b, hf * HD:(hf + 1) * HD])
                        nc.gpsimd.indirect_dma_start(
                            out=outs[hf][:],
                            out_offset=bass.IndirectOffsetOnAxis(
                                ap=ossb[:, eli * NB + b:eli * NB + b + 1],
                                axis=0),
                            in_=yo[:], in_offset=None)

                rf = {}
                XT = {}
                ht = {}
                for el in range(EPC):
                    wk_sb = pwk.tile([P, DC, F], BF16, tag="wk")
                    nc.sync.dma_start(
                        out=wk_sb[:],
                        in_=wk[el].rearrange("(c p) f -> p c f", p=P))
                    wv_sb = pwv.tile([P, FC, D], BF16, tag="wv")
                    nc.sync.dma_start(
                        out=wv_sb[:],
                        in_=wv[el].rearrange("(c p) f -> p c f", p=P))
                    for ck in range(2):
                        XT[ck] = pfx.tile([P, DC, 512], BF16, tag=f"XT{ck}",
                                          name=f"XT{ck}")
                        col0 = (el * C + ck * 512) // 16
                        nc.gpsimd.dma_gather(
                            out_ap=XT[ck][:], in_ap=recv1[:],
                            idxs_ap=sl16[:, col0:col0 + 32],
                            num_idxs=512, num_idxs_reg=512, elem_size=D,
                            transpose=True)
                    # FFN1 per chunk
                    for ck in range(2):
                        ht[ck] = pfh.tile([P, FC, 512], BF16, tag=f"ht{ck}",
                                          name=f"ht{ck}")
                        for ft in range(FC):
                            ph = psh.tile([P, 512], F32, space="PSUM",
                                          tag="ph")
                            for dc in range(DC):
                                nc.tensor.matmul(
                                    out=ph[:],
                                    lhsT=wk_sb[:, dc, ft * P:(ft + 1) * P],
                                    rhs=XT[ck][:, dc, :],
                                    start=(dc == 0), stop=(dc == DC - 1))
                            hr = pfhr.tile([P, 512], BF16, tag="hr")
                            nc.scalar.activation(out=hr[:], in_=ph[:],
                                                 func=AF.Relu)
                            nc.vector.tensor_mul(out=ht[ck][:, ft, :],
                                                 in0=hr[:], in1=hr[:])
                        if el == 1 and ck == 0:
                            d_chunk(0, 0)        # A2A(0,0) long done
                        if el == 1 and ck == 1:
                            # r rows for el=1 (r_dram complete since
                            # receptance); rf[0] freed after d_chunk(0,1)
                            d_chunk(0, 1)
                    rf[el] = prf.tile([P, NB, D], BF16, tag="rf",
                                      name=f"rf{el}")
                    base = el * (NBR // 16)
                    for s0 in range(0, NB, 4):
                        s1 = min(s0 + 4, NB)
                        nseg = (s1 - s0) * P
                        nc.gpsimd.dma_gather(
                            out_ap=rf[el][:, s0:s1, :], in_ap=r_dram[:],
                            idxs_ap=rgsb[:, base + s0 * 8:base + s1 * 8],
                            num_idxs=nseg, num_idxs_reg=nseg, elem_size=D,
                            transpose=False)
                    # FFN2, output-column-half major; A2A per half
                    for hf in range(2):
                        for ck in range(2):
                            for tt in range(4):
                                py = psy.tile([P, HD], F32, space="PSUM",
                                              tag="py")
                                for fc in range(FC):
                                    nc.tensor.matmul(
                                        out=py[:],
                                        lhsT=ht[ck][:, fc,
                                                    tt * P:(tt + 1) * P],
                                        rhs=wv_sb[:, fc,
                                                  hf * HD:(hf + 1) * HD],
                                        start=(fc == 0), stop=(fc == FC - 1))
                                ysb = pfy.tile([P, HD], BF16, tag="ysb")
                                nc.vector.tensor_copy(out=ysb[:], in_=py[:])
                                scol = el * 8 + ck * 4 + tt
                                nc.gpsimd.indirect_dma_start(
                                    out=a2h[el][hf][:],
                                    out_offset=bass.IndirectOffsetOnAxis(
                                        ap=s2sb[:, scol:scol + 1], axis=0),
                                    in_=ysb[:], in_offset=None)
                        nc.gpsimd.collective_compute(
                            "AllToAll", mybir.AluOpType.bypass,
                            replica_groups=rg,
                            ins=[a2h[el][hf][0:NBR, :]],
                            outs=[recv2h[el][hf][0:NBR, :]])
                d_chunk(1, 0)
                d_chunk(1, 1)

    nc.finalize()
    return nc


def _prepare_inputs(x, token_ids, shift_state, time_maa_k, time_maa_r,
                    w_recept, w_key, w_value):
    cfg, idxs = _build_indices(token_ids)
    x = np.asarray(x, np.float32)
    shift = np.asarray(shift_state, np.float32)
    wrt = np.ascontiguousarray(np.asarray(w_recept, np.float32).T).astype(nbf16)
    wkb = np.asarray(w_key, np.float32).astype(nbf16)
    wvb = np.asarray(w_value, np.float32).astype(nbf16)
    mk = np.asarray(time_maa_k, np.float32)[None, :].astype(nbf16)
    mr = np.asarray(time_maa_r, np.float32)[None, :].astype(nbf16)
    iota = np.tile(np.arange(512, dtype=np.int16).reshape(-1, 16).T, (8, 1))

    in_maps = []
    for k in range(NCORES):
        x_ext = np.concatenate([shift[k:k + 1], x[k]], axis=0)
        in_maps.append({
            "x_ext": np.ascontiguousarray(x_ext),
            "maa_k": mk, "maa_r": mr, "wrt": wrt,
            "wk": np.ascontiguousarray(wkb[EPC * k:EPC * (k + 1)]),
            "wv": np.ascontiguousarray(wvb[EPC * k:EPC * (k + 1)]),
            "iota16": iota,
            **idxs[k],
        })
    return cfg, in_maps


def kernel(x, token_ids, shift_state, time_maa_k, time_maa_r,
           w_recept, w_key, w_value, _trace=False):
    cfg, in_maps = _prepare_inputs(x, token_ids, shift_state, time_maa_k,
                                   time_maa_r, w_recept, w_key, w_value)
    if cfg not in _CACHE:
        _CACHE[cfg] = _build_nc(cfg)
    nc = _CACHE[cfg]
    res = run_bass_kernel_spmd(nc, in_maps, core_ids=list(range(NCORES)),
                               trace=_trace)
    kernel.last_result = res
    y = np.stack(
        [np.concatenate([res.results[k]["out0"][:T], res.results[k]["out1"][:T]],
                        axis=1) for k in range(NCORES)], axis=0)
    return y.astype(np.float32)


# revision 9
# speedup vs baseline: 1.0898x; 1.0898x over previous
"""Expert-parallel CMoE kernel for 8 Trainium2 NeuronCores (v2).

Sharding (hardcoded for B=8, T=2048, D=1024, F=2048, E=16, C=1024):
  core k owns batch k (token shift, receptance, output) and experts
  {2k, 2k+1} (FFN). Hash routing is int math on token_ids, done on host;
  the resulting permutations ship to the cores as index tensors.

Schedule per core (PE-centric; everything else hides behind matmuls):
  phase A  (~35us, PE idle, DMA/DVE-bound): stream x once per tile,
           token-shift via partition-offset copies (no second HBM load),
           all-bf16 vector math; scatter xk rows into the single
           dispatch buffer; store xr chunks for the receptance phase.
  dispatch: ONE AllToAll (4.7 MB) fired at loop end; it runs on the
           collective engine while the PE does the receptance.
  receptance (~68us PE): per 512-token chunk, transposing dma_gather of
           xr, 64 matmuls, sigmoid, store r rows to DRAM.
  phase C (~275us PE): per expert el in {0,1}: transposing gathers of
           the expert queue, FFN1 (relu^2), FFN2 split by output
           column-half; each (el, half) fires its own combine AllToAll
           (1.45 MB) so only the last one is exposed.
  phase D  (interleaved): per (el, half): contiguous load of the recv
           buffer, multiply by gathered r rows, indirect-scatter fp32
           rows into out[token]. Outputs are two [T+1, 512] tensors
           (indirect DMA needs offset-0 APs); row T is a trash row for
           pad slots; dropped tokens keep the zero-init value.
All matmuls bf16 with fp32 PSUM accumulation.
"""
import sys

for _p in ("/opt/trn_rl_repo", "/root/.axon_site/_ro/trn_rl_repo"):
    if _p not in sys.path:
        sys.path.append(_p)

import numpy as np
import ml_dtypes

import concourse.bass as bass
import concourse.bacc as bacc
import concourse.mybir as mybir
import concourse.tile as tile
from concourse.bass_utils import run_bass_kernel_spmd

P = 128
B, T, D, F, E = 8, 2048, 1024, 2048, 16
N = B * T
C = max(4, N // E)          # 1024
HASH_PRIME = 5099
NCORES = 8
EPC = E // NCORES           # experts per core = 2
DC = D // P                 # 8
FC = F // P                 # 16
BF16 = mybir.dt.bfloat16
F32 = mybir.dt.float32
I16 = mybir.dt.int16
I32 = mybir.dt.int32
nbf16 = ml_dtypes.bfloat16
AF = mybir.ActivationFunctionType

_CACHE = {}


def _r16(v):
    return int(-(-int(v) // 16) * 16)


def _wrap16(a):
    a = np.asarray(a, np.int16)
    w = a.reshape(-1, 16).T.copy()       # j at [j%16, j//16]
    return np.tile(w, (8, 1))            # replicated across 8 Q7 cores


def _route(token_ids):
    tid = np.asarray(token_ids).reshape(N).astype(np.int64)
    e = (tid * HASH_PRIME) % E
    onehot = (e[:, None] == np.arange(E)).astype(np.int64)
    pos = onehot.cumsum(0)[np.arange(N), e] - 1
    keep = pos < C
    return e, pos, keep


def _build_indices(token_ids):
    e, pos, keep = _route(token_ids)
    n_idx = np.arange(N)
    src = n_idx // T
    dst = e // EPC
    el = e % EPC
    kept = n_idx[keep]

    # dispatch: single A2A; rank within (src,dst) in n order
    cntd = np.zeros((NCORES, NCORES), np.int64)
    rankd = np.zeros(N, np.int64)
    for n in kept:
        rankd[n] = cntd[src[n], dst[n]]
        cntd[src[n], dst[n]] += 1
    KD = _r16(cntd.max())
    ZRD = NCORES * KD
    slot = np.where(keep, dst * KD + rankd, ZRD)

    recv_row = np.full((NCORES, EPC * C), ZRD, np.int64)
    for n in kept:
        recv_row[dst[n], el[n] * C + pos[n]] = src[n] * KD + rankd[n]

    # combine: 2 row-chunks by el; A2As additionally split by col half
    cellcnt = np.zeros((NCORES, NCORES, EPC), np.int64)
    rankc = np.zeros(N, np.int64)
    for n in kept:
        rankc[n] = cellcnt[dst[n], src[n], el[n]]
        cellcnt[dst[n], src[n], el[n]] += 1
    KC = _r16(cellcnt.max())
    NBR = NCORES * KC
    ZRC = NBR

    s2 = np.full((NCORES, EPC, C), ZRC, np.int64)
    for n in kept:
        s2[dst[n], el[n], pos[n]] = src[n] * KC + rankc[n]

    rg_idx = np.zeros((NCORES, EPC, NBR), np.int64)
    os_idx = np.full((NCORES, EPC, NBR), T, np.int64)
    for n in kept:
        h = src[n]
        lt = n - h * T
        row = dst[n] * KC + rankc[n]
        rg_idx[h, el[n], row] = lt
        os_idx[h, el[n], row] = lt

    NB = NBR // P
    per_core = []
    for k in range(NCORES):
        tok = slice(k * T, (k + 1) * T)
        per_core.append({
            "srcD32": slot[tok].astype(np.int32).reshape(T // P, P).T.copy(),
            "slot16": _wrap16(recv_row[k]),
            "s2_32": np.concatenate(
                [s2[k, eli].reshape(C // P, P).T for eli in range(EPC)],
                axis=1).astype(np.int32).copy(),
            "rg16": np.concatenate(
                [_wrap16(rg_idx[k, eli]) for eli in range(EPC)], axis=1),
            "os32": np.concatenate(
                [os_idx[k, eli].reshape(NB, P).T for eli in range(EPC)],
                axis=1).astype(np.int32).copy(),
        })
    return (KD, KC), per_core


def _build_nc(cfg):
    KD, KC = cfg
    RD = NCORES * KD
    NBR = NCORES * KC
    NB = NBR // P
    HD = D // 2              # 512, output column half

    nc = bacc.Bacc("TRN2", target_bir_lowering=False, debug=False,
                   num_devices=NCORES)

    x_ext = nc.dram_tensor("x_ext", [T + 1, D], F32, kind="ExternalInput")
    maa_k = nc.dram_tensor("maa_k", [1, D], BF16, kind="ExternalInput")
    maa_r = nc.dram_tensor("maa_r", [1, D], BF16, kind="ExternalInput")
    wrt = nc.dram_tensor("wrt", [D, D], BF16, kind="ExternalInput")
    wk = nc.dram_tensor("wk", [EPC, D, F], BF16, kind="ExternalInput")
    wv = nc.dram_tensor("wv", [EPC, F, D], BF16, kind="ExternalInput")
    srcD32 = nc.dram_tensor("srcD32", [P, T // P], I32, kind="ExternalInput")
    slot16 = nc.dram_tensor("slot16", [P, EPC * C // 16], I16,
                            kind="ExternalInput")
    s2_32 = nc.dram_tensor("s2_32", [P, EPC * (C // P)], I32,
                           kind="ExternalInput")
    rg16 = nc.dram_tensor("rg16", [P, EPC * (NBR // 16)], I16,
                          kind="ExternalInput")
    os32 = nc.dram_tensor("os32", [P, EPC * NB], I32, kind="ExternalInput")
    iota16 = nc.dram_tensor("iota16", [P, 512 // 16], I16,
                            kind="ExternalInput")
    out0 = nc.dram_tensor("out0", [T + 1, HD], F32, kind="ExternalOutput")
    out1 = nc.dram_tensor("out1", [T + 1, HD], F32, kind="ExternalOutput")
    outs = [out0, out1]

    rg = [list(range(NCORES))]

    with tile.TileContext(nc) as tc:
        with (
            tc.tile_pool(name="dram", bufs=1, space="DRAM") as dram,
            tc.tile_pool(name="misc", bufs=1) as misc,
        ):
            a1_in = dram.tile([RD + 1, D], BF16)
            recv1 = dram.tile([RD + 1, D], BF16)
            a2h = [[dram.tile([NBR + 1, HD], BF16, name=f"a2_{eli}_{hf}")
                    for hf in range(2)] for eli in range(EPC)]
            recv2h = [[dram.tile([NBR, HD], BF16, name=f"rc2_{eli}_{hf}")
                       for hf in range(2)] for eli in range(EPC)]
            xr_bufs = [dram.tile([512, D], BF16, name=f"xr_buf{i}")
                       for i in range(4)]
            r_dram = dram.tile([T, D], BF16)

            # phase-A-critical loads only; everything else is deferred
            srcDsb = misc.tile([P, T // P], I32)
            nc.sync.dma_start(out=srcDsb[:], in_=srcD32[:])

            with tc.tile_pool(name="pw0", bufs=1) as pw0:
                maakb = pw0.tile([P, D], BF16)
                nc.sync.dma_start(out=maakb[:],
                                  in_=maa_k[:].to_broadcast([P, D]))
                maarb = pw0.tile([P, D], BF16)
                nc.sync.dma_start(out=maarb[:],
                                  in_=maa_r[:].to_broadcast([P, D]))

                # ---- phase A: token shift, dispatch scatter, xr spill.
                # DVE: sub/mul/add for xk; Pool: mul/add for xr; xk rows
                # batched 4 tiles per indirect scatter.
                with tc.tile_pool(name="pa", bufs=4) as pa:
                    for t in range(T // P):
                        xc = pa.tile([P, D], F32, tag="xc")
                        nc.sync.dma_start(
                            out=xc[:],
                            in_=x_ext[1 + t * P:1 + (t + 1) * P, :])
                        xp = pa.tile([P, D], F32, tag="xp")
                        nc.sync.dma_start(
                            out=xp[:], in_=x_ext[t * P:(t + 1) * P, :])
                        xcb = pa.tile([P, D], BF16, tag="xcb")
                        nc.scalar.activation(out=xcb[:], in_=xc[:],
                                             func=AF.Copy)
                        dx = pa.tile([P, D], BF16, tag="dx")
                        nc.vector.tensor_sub(out=dx[:], in0=xp[:], in1=xc[:])
                        tmpk = pa.tile([P, D], BF16, tag="tmpk")
                        nc.vector.tensor_mul(out=tmpk[:], in0=dx[:],
                                             in1=maakb[:])
                        xk = pa.tile([P, D], BF16, tag="xk")
                        nc.vector.tensor_add(out=xk[:], in0=tmpk[:],
                                             in1=xcb[:])
                        nc.gpsimd.indirect_dma_start(
                            out=a1_in[:],
                            out_offset=bass.IndirectOffsetOnAxis(
                                ap=srcDsb[:, t:t + 1], axis=0),
                            in_=xk[:], in_offset=None)
                        tmpr = pa.tile([P, D], BF16, tag="tmpr")
                        nc.gpsimd.tensor_mul(out=tmpr[:], in0=dx[:],
                                             in1=maarb[:])
                        xr = pa.tile([P, D], BF16, tag="xr")
                        nc.gpsimd.tensor_add(out=xr[:], in0=tmpr[:],
                                             in1=xcb[:])
                        nc.scalar.dma_start(
                            out=xr_bufs[t // 4][(t % 4) * P:(t % 4 + 1) * P, :],
                            in_=xr[:])

                # ---- dispatch A2A trigger first; its ~20us entry barrier
                # gives the xrT gathers a contention-free window
                nc.gpsimd.collective_compute(
                    "AllToAll", mybir.AluOpType.bypass, replica_groups=rg,
                    ins=[a1_in[0:RD, :]], outs=[recv1[0:RD, :]])

                # deferred loads (sync queue is idle now)
                io16 = misc.tile([P, 512 // 16], I16)
                nc.sync.dma_start(out=io16[:], in_=iota16[:])
                wrt_sb = pw0.tile([P, DC, D], BF16)
                nc.sync.dma_start(
                    out=wrt_sb[:], in_=wrt.rearrange("(c p) e -> p c e", p=P))
                sl16 = misc.tile([P, EPC * C // 16], I16)
                nc.sync.dma_start(out=sl16[:], in_=slot16[:])
                s2sb = misc.tile([P, EPC * (C // P)], I32)
                nc.sync.dma_start(out=s2sb[:], in_=s2_32[:])
                rgsb = misc.tile([P, EPC * (NBR // 16)], I16)
                nc.sync.dma_start(out=rgsb[:], in_=rg16[:])
                ossb = misc.tile([P, EPC * NB], I32)
                nc.sync.dma_start(out=ossb[:], in_=os32[:])
                zrow = pw0.tile([1, D], BF16)
                nc.vector.memzero(zrow[:])
                nc.scalar.dma_start(out=recv1[RD:RD + 1, :], in_=zrow[:])

                # ---- receptance: all 4 gathers prefetched up front
                with (
                    tc.tile_pool(name="prx", bufs=4) as prx,
                    tc.tile_pool(name="prs", bufs=2) as prs,
                    tc.tile_pool(name="psr", bufs=2, space="PSUM") as psr,
                ):
                    xrTs = []
                    for ck in range(4):
                        xrT = prx.tile([P, DC, 512], BF16, tag="xrT",
                                       name=f"xrT{ck}")
                        nc.gpsimd.dma_gather(
                            out_ap=xrT[:], in_ap=xr_bufs[ck][:],
                            idxs_ap=io16[:, 0:32],
                            num_idxs=512, num_idxs_reg=512, elem_size=D,
                            transpose=True)
                        xrTs.append(xrT)
                    for ck in range(4):
                        xrT = xrTs[ck]
                        for tt in range(4):
                            pr0 = psr.tile([P, 512], F32, space="PSUM",
                                           tag="pr0")
                            pr1 = psr.tile([P, 512], F32, space="PSUM",
                                           tag="pr1")
                            for dc in range(DC):
                                nc.tensor.matmul(
                                    out=pr0[:],
                                    lhsT=xrT[:, dc, tt * P:(tt + 1) * P],
                                    rhs=wrt_sb[:, dc, 0:512],
                                    start=(dc == 0), stop=(dc == DC - 1))
                                nc.tensor.matmul(
                                    out=pr1[:],
                                    lhsT=xrT[:, dc, tt * P:(tt + 1) * P],
                                    rhs=wrt_sb[:, dc, 512:1024],
                                    start=(dc == 0), stop=(dc == DC - 1))
                            rsb = prs.tile([P, D], BF16, tag="rsb")
                            nc.scalar.activation(out=rsb[:, 0:512],
                                                 in_=pr0[:], func=AF.Sigmoid)
                            nc.scalar.activation(out=rsb[:, 512:1024],
                                                 in_=pr1[:], func=AF.Sigmoid)
                            r0 = ck * 512 + tt * P
                            nc.scalar.dma_start(out=r_dram[r0:r0 + P, :],
                                                in_=rsb[:])

            # ---------------- phase C: expert FFNs + combine + phase D
            with (
                tc.tile_pool(name="pwk", bufs=2) as pwk,
                tc.tile_pool(name="pwv", bufs=1) as pwv,
                tc.tile_pool(name="pfx", bufs=1) as pfx,
                tc.tile_pool(name="pfh", bufs=1) as pfh,
                tc.tile_pool(name="pfhr", bufs=1) as pfhr,
                tc.tile_pool(name="pfy", bufs=2) as pfy,
                tc.tile_pool(name="prf", bufs=1) as prf,
                tc.tile_pool(name="pdl", bufs=1) as pdl,
                tc.tile_pool(name="pdo", bufs=1) as pdo,
                tc.tile_pool(name="psh", bufs=2, space="PSUM") as psh,
                tc.tile_pool(name="psy", bufs=2, space="PSUM") as psy,
            ):
                def d_chunk(eli, hf):
                    """recv2h[eli][hf] -> out[hf] rows (after its A2A)."""
                    yrecv = pdl.tile([P, NB, HD], BF16, tag="yrecv")
                    nc.sync.dma_start(
                        out=yrecv[:],
                        in_=recv2h[eli][hf].rearrange("(b p) d -> p b d", p=P))
                    for s0 in range(0, NB, 6):
                        s1 = min(s0 + 6, NB)
                        yo = pdo.tile([P, 6, HD], F32, tag="yo")
                        nc.vector.tensor_mul(
                            out=yo[:, 0:s1 - s0, :],
                            in0=yrecv[:, s0:s1, :],
                            in1=rf[eli][:, s0:s1, hf * HD:(hf + 1) * HD])
                        for b in range(s0, s1):
                            nc.gpsimd.indirect_dma_start(
                                out=outs[hf][:],
                                out_offset=bass.IndirectOffsetOnAxis(
                                    ap=ossb[:, eli * NB + b:eli * NB + b + 1],
                                    axis=0),
                                in_=yo[:, b - s0, :], in_offset=None)

                def rf_gather(eli):
                    rf[eli] = prf.tile([P, NB, D], BF16, tag="rf",
                                       name=f"rf{eli}")
                    base = eli * (NBR // 16)
                    for s0 in range(0, NB, 4):
                        s1 = min(s0 + 4, NB)
                        nseg = (s1 - s0) * P
                        nc.gpsimd.dma_gather(
                            out_ap=rf[eli][:, s0:s1, :], in_ap=r_dram[:],
                            idxs_ap=rgsb[:, base + s0 * 8:base + s1 * 8],
                            num_idxs=nseg, num_idxs_reg=nseg, elem_size=D,
                            transpose=False)

                rf = {}
                XT = {}
                ht = {}
                for el in range(EPC):
                    wk_sb = pwk.tile([P, DC, F], BF16, tag="wk")
                    nc.sync.dma_start(
                        out=wk_sb[:],
                        in_=wk[el].rearrange("(c p) f -> p c f", p=P))
                    wv_sb = pwv.tile([P, FC, D], BF16, tag="wv")
                    nc.sync.dma_start(
                        out=wv_sb[:],
                        in_=wv[el].rearrange("(c p) f -> p c f", p=P))
                    for ck in range(2):
                        XT[ck] = pfx.tile([P, DC, 512], BF16, tag=f"XT{ck}",
                                          name=f"XT{ck}")
                        col0 = (el * C + ck * 512) // 16
                        nc.gpsimd.dma_gather(
                            out_ap=XT[ck][:], in_ap=recv1[:],
                            idxs_ap=sl16[:, col0:col0 + 32],
                            num_idxs=512, num_idxs_reg=512, elem_size=D,
                            transpose=True)
                    if el == 0:
                        rf_gather(0)    # runs during FFN1(e0) on gpsimd
                    # FFN1 per chunk
                    for ck in range(2):
                        ht[ck] = pfh.tile([P, FC, 512], BF16, tag=f"ht{ck}",
                                          name=f"ht{ck}")
                        for ft in range(FC):
                            ph = psh.tile([P, 512], F32, space="PSUM",
                                          tag="ph")
                            for dc in range(DC):
                                nc.tensor.matmul(
                                    out=ph[:],
                                    lhsT=wk_sb[:, dc, ft * P:(ft + 1) * P],
                                    rhs=XT[ck][:, dc, :],
                                    start=(dc == 0), stop=(dc == DC - 1))
                            hr = pfhr.tile([P, 512], BF16, tag="hr")
                            nc.scalar.activation(out=hr[:], in_=ph[:],
                                                 func=AF.Relu)
                            nc.vector.tensor_mul(out=ht[ck][:, ft, :],
                                                 in0=hr[:], in1=hr[:])
                        if el == 1 and ck == 0:
                            d_chunk(0, 0)        # A2A(0,0) long done
                        if el == 1 and ck == 1:
                            d_chunk(0, 1)
                    if el == 1:
                        rf_gather(1)    # after d_chunk(0,1) frees rf buffer
                    # FFN2, output-column-half major; A2A per half
                    for hf in range(2):
                        for ck in range(2):
                            for tt in range(4):
                                py = psy.tile([P, HD], F32, space="PSUM",
                                              tag="py")
                                for fc in range(FC):
                                    nc.tensor.matmul(
                                        out=py[:],
                                        lhsT=ht[ck][:, fc,
                                                    tt * P:(tt + 1) * P],
                                        rhs=wv_sb[:, fc,
                                                  hf * HD:(hf + 1) * HD],
                                        start=(fc == 0), stop=(fc == FC - 1))
                                ysb = pfy.tile([P, HD], BF16, tag="ysb")
                                nc.vector.tensor_copy(out=ysb[:], in_=py[:])
                                scol = el * 8 + ck * 4 + tt
                                nc.gpsimd.indirect_dma_start(
                                    out=a2h[el][hf][:],
                                    out_offset=bass.IndirectOffsetOnAxis(
                                        ap=s2sb[:, scol:scol + 1], axis=0),
                                    in_=ysb[:], in_offset=None)
                        nc.gpsimd.collective_compute(
                            "AllToAll", mybir.AluOpType.bypass,
                            replica_groups=rg,
                            ins=[a2h[el][hf][0:NBR, :]],
                            outs=[recv2h[el][hf][0:NBR, :]])
                d_chunk(1, 0)
                d_chunk(1, 1)

    nc.finalize()
    return nc


def _prepare_inputs(x, token_ids, shift_state, time_maa_k, time_maa_r,
                    w_recept, w_key, w_value):
    cfg, idxs = _build_indices(token_ids)
    x = np.asarray(x, np.float32)
    shift = np.asarray(shift_state, np.float32)
    wrt = np.ascontiguousarray(np.asarray(w_recept, np.float32).T).astype(nbf16)
    wkb = np.asarray(w_key, np.float32).astype(nbf16)
    wvb = np.asarray(w_value, np.float32).astype(nbf16)
    mk = np.asarray(time_maa_k, np.float32)[None, :].astype(nbf16)
    mr = np.asarray(time_maa_r, np.float32)[None, :].astype(nbf16)
    iota = np.tile(np.arange(512, dtype=np.int16).reshape(-1, 16).T, (8, 1))

    in_maps = []
    for k in range(NCORES):
        x_ext = np.concatenate([shift[k:k + 1], x[k]], axis=0)
        in_maps.append({
            "x_ext": np.ascontiguousarray(x_ext),
            "maa_k": mk, "maa_r": mr, "wrt": wrt,
            "wk": np.ascontiguousarray(wkb[EPC * k:EPC * (k + 1)]),
            "wv": np.ascontiguousarray(wvb[EPC * k:EPC * (k + 1)]),
            "iota16": iota,
            **idxs[k],
        })
    return cfg, in_maps


def kernel(x, token_ids, shift_state, time_maa_k, time_maa_r,
           w_recept, w_key, w_value, _trace=False):
    cfg, in_maps = _prepare_inputs(x, token_ids, shift_state, time_maa_k,
                                   time_maa_r, w_recept, w_key, w_value)
    if cfg not in _CACHE:
        _CACHE[cfg] = _build_nc(cfg)
    nc = _CACHE[cfg]
    res = run_bass_kernel_spmd(nc, in_maps, core_ids=list(range(NCORES)),
                               trace=_trace)
    kernel.last_result = res
    y = np.stack(
        [np.concatenate([res.results[k]["out0"][:T], res.results[k]["out1"][:T]],
                        axis=1) for k in range(NCORES)], axis=0)
    return y.astype(np.float32)
